# revision 12
# baseline (speedup 1.0000x reference)
"""Trainium2 Bass kernel for the AutoregressiveSplineDeep flow (tabulated).

The MADE mask structure makes dim-0's spline parameters constant and dim-1's
parameters a function of the scalar z0 only, and every flow step applies the
same transform.  So the kernel tabulates (on host, from the weights only):
  - D0: the four composed dim-0 maps T^s as piecewise-linear tables of x0
        (value + slope per cell), since z0^(s) = T^s(x0);
  - DZ: per (z0-grid row j, z1-cell): bin AND knot-side resolved at the
        cell center (the spline is C1, so near-boundary mis-selection only
        extrapolates the adjacent C1 piece), with the linear-rational
        coefficients pre-composed in xc: y = (A + B*xc)/(G + D*xc), stored
        for grid rows j and j+1 (lerp in z0).

On device each flow step for dim 1 is ONE data-dependent lookup per sample:
grid/cell index arithmetic -> ap_gather (per-core wrapped column gather,
the 8 coefficients living on the core's 16 partitions) -> PE transposes
back to sample-major -> lerp in z0 -> one rational eval.  No matmuls or
activations in the hot path.  Data-parallel over 8 NeuronCores (16384
samples each); the Q7 ap_gather cost (~27.5 ns/index, ~56 us per
2048-index wave) dominates: 4 sequential z1 waves + 1 dim-0 wave.

Sample layout per core: sample (c, r0, u) <-> partition u, free (r0, c),
with c = Q7 core group, chosen so one PE transpose maps index tiles into
ap_gather's wrapped index layout and strided PE transposes map gather
results back.
"""

import sys

sys.path.insert(0, "/opt/trn_rl_repo")

import numpy as np

INPUT_DIM = 2
K = 16
BOUND = 5.0
FLOW_LENGTH = 4
MIN_BIN = 1e-3
MIN_DERIV = 1e-3
MIN_LAMBDA = 0.025
LEFT, RIGHT = -BOUND, BOUND
N_FULL = 131072
N_CORES = 8
NS = N_FULL // N_CORES

M = 64            # z0 grid rows (lerp)
M1 = 256          # z1 cells per grid row
M0 = 4096         # x0 cells for the dim-0 chain tables
GLO, GHI = -5.3, 5.3
X0LO, X0HI = -5.44, 5.44
R1 = M * M1       # T1C rows
R2 = 32 * M       # T2 rows


# --------------------------------------------------------------------------
# host-side table construction (float64)
# --------------------------------------------------------------------------

def _mlp_dim1(z0, W1, b1, W2, b2, W3, b3):
    z0 = np.asarray(z0, np.float64)
    h1 = np.maximum(np.outer(z0, W1[:, 0].astype(np.float64)) + b1, 0.0)
    h2 = np.maximum(h1 @ W2.T.astype(np.float64) + b2, 0.0)
    return h2 @ W3[1::2].T.astype(np.float64) + b3[1::2]


def _process_tables(raw):
    """raw [G, 63] -> per-bin spline tables + rational coefficients."""
    G = raw.shape[0]
    w, h = raw[:, 0:K], raw[:, K:2 * K]
    dd, ll = raw[:, 2 * K:3 * K - 1], raw[:, 3 * K - 1:4 * K - 1]

    def smax(v):
        e = np.exp(v - v.max(-1, keepdims=True))
        return e / e.sum(-1, keepdims=True)

    widths = MIN_BIN + (1 - MIN_BIN * K) * smax(w)
    cw = np.concatenate([np.zeros((G, 1)), np.cumsum(widths, -1)], -1)
    cw = (RIGHT - LEFT) * cw + LEFT
    cw[:, 0], cw[:, -1] = LEFT, RIGHT
    widths = np.diff(cw, axis=-1)
    heights = MIN_BIN + (1 - MIN_BIN * K) * smax(h)
    ch = np.concatenate([np.zeros((G, 1)), np.cumsum(heights, -1)], -1)
    ch = (RIGHT - LEFT) * ch + LEFT
    ch[:, 0], ch[:, -1] = LEFT, RIGHT
    heights = np.diff(ch, axis=-1)
    deriv = np.concatenate(
        [np.ones((G, 1)), MIN_DERIV + np.log1p(np.exp(dd)), np.ones((G, 1))],
        -1)
    lam = MIN_LAMBDA + (1 - 2 * MIN_LAMBDA) * (1 / (1 + np.exp(-ll)))

    xk, wk, yk, hk = cw[:, :K], widths, ch[:, :K], heights
    dk, dk1 = deriv[:, :K], deriv[:, 1:K + 1]
    wb = np.sqrt(dk / dk1)
    wc = (lam * dk + (1 - lam) * wb * dk1) * (wk / hk)
    ya, yb = yk, hk + yk
    yc = ((1 - lam) * ya + lam * wb * yb) / ((1 - lam) + lam * wb)
    aL, bL, gL, dL = ya * lam, wc * yc - ya, lam * np.ones_like(ya), wc - 1.0
    aR, bR = wc * yc - wb * yb * lam, wb * yb - wc * yc
    gR, dR = wc - wb * lam, wb - wc
    return dict(cw=cw, xk=xk, invwk=1.0 / wk, iw=1.0 / wk, lam=lam, yk=yk,
                ya=ya, yb=yb, yc=yc, wb=wb, wc=wc,
                aL=aL - yk * gL, bL=bL - yk * dL, gL=gL, dL=dL,
                aR=aR - yk * gR, bR=bR - yk * dR, gR=gR, dR=dR)


def _coefs_xc(t, j, b, side):
    """Unshifted rational coefficients composed in xc:
    y = (A + B*xc)/(G + D*xc)."""
    xk, iw, lam = t["xk"][j, b], t["iw"][j, b], t["lam"][j, b]
    wb, wc = t["wb"][j, b], t["wc"][j, b]
    ya, yb, yc = t["ya"][j, b], t["yb"][j, b], t["yc"][j, b]
    if side == 0:
        al, be, ga, de = ya * lam, wc * yc - ya, lam, wc - 1.0
    else:
        al = wc * yc - wb * yb * lam
        be = wb * yb - wc * yc
        ga, de = wc - wb * lam, wb - wc
    return al - be * iw * xk, be * iw, ga - de * iw * xk, de * iw


def _spline_exact(z, t, g=0):
    z = np.asarray(z, np.float64)
    xc = np.clip(z, LEFT, RIGHT)
    b = np.clip(np.searchsorted(t["cw"][g, 1:K], xc, side="right"), 0, K - 1)
    th = (xc - t["xk"][g, b]) * t["invwk"][g, b]
    s = th > t["lam"][g, b]
    a = np.where(s, t["aR"][g, b], t["aL"][g, b])
    bb = np.where(s, t["bR"][g, b], t["bL"][g, b])
    gg = np.where(s, t["gR"][g, b], t["gL"][g, b])
    dd = np.where(s, t["dR"][g, b], t["dL"][g, b])
    y = t["yk"][g, b] + (a + bb * th) / (gg + dd * th)
    return np.where((z >= LEFT) & (z <= RIGHT), y, z)


def _build_tables(W1, b1, W2, b2, W3, b3):
    f32 = np.float32
    # dim-0: constant spline from even outputs (raw = b3 even)
    tabs0 = _process_tables(b3[0::2].astype(np.float64)[None, :])
    edges = X0LO + (X0HI - X0LO) * np.arange(M0 + 1) / M0
    cur = edges.copy()
    v0 = np.empty((4, M0 + 1))
    for s in range(4):
        cur = _spline_exact(cur, tabs0)
        v0[s] = cur
    D0 = np.zeros((16, M0), f32)
    for s in range(4):
        D0[2 * s] = v0[s, :M0]
        D0[2 * s + 1] = v0[s, 1:] - v0[s, :-1]

    # dim-1: z0-grid tables, bin AND side resolved per (j, z1-cell),
    # coefficients composed in xc: y = (A + B*xc)/(G + D*xc)
    zg = GLO + (GHI - GLO) * np.arange(M) / (M - 1)
    t1 = _process_tables(_mlp_dim1(zg, W1, b1, W2, b2, W3, b3))
    cellx = np.arange(M1) / (M1 / (2 * BOUND)) - BOUND
    jp = np.minimum(np.arange(M) + 1, M - 1)
    DZ = np.zeros((16, R1), f32)
    for j in range(M):
        b = np.clip(
            np.searchsorted(t1["cw"][j, 1:K], cellx, side="right"), 0, K - 1)
        th_c = (cellx - t1["xk"][j, b]) * t1["iw"][j, b]
        side = (th_c > t1["lam"][j, b]).astype(int)
        sl = slice(j * M1, (j + 1) * M1)
        for v, jj in enumerate([j, jp[j]]):
            for si in (0, 1):
                m = side == si
                A, B, G, D = _coefs_xc(t1, jj, b[m], si)
                DZ[4 * v + 0, sl][m] = A
                DZ[4 * v + 1, sl][m] = B
                DZ[4 * v + 2, sl][m] = G
                DZ[4 * v + 3, sl][m] = D
    return D0, DZ


# --------------------------------------------------------------------------
# bass program
# --------------------------------------------------------------------------

def _build_program(ns):
    import concourse.bacc as bacc
    import concourse.tile as tile
    import concourse.mybir as mybir

    F32 = mybir.dt.float32
    I16 = mybir.dt.int16
    U8 = mybir.dt.uint8
    ALU = mybir.AluOpType

    nc = bacc.Bacc("TRN2", target_bir_lowering=False, debug=False,
                   num_devices=N_CORES)

    xa = nc.dram_tensor("XA", [128, 16, 8, 2], F32, kind="ExternalInput").ap()
    d0 = nc.dram_tensor("D0", [128, M0], F32, kind="ExternalInput").ap()
    dz = nc.dram_tensor("DZ", [128, R1], F32, kind="ExternalInput").ap()
    ident = nc.dram_tensor("IDENT", [128, 128], F32, kind="ExternalInput").ap()
    z0out = nc.dram_tensor("Z0OUT", [FLOW_LENGTH, 128, 16, 8], F32,
                           kind="ExternalOutput").ap()
    z1out = nc.dram_tensor("Z1OUT", [FLOW_LENGTH, 128, 16, 8], F32,
                           kind="ExternalOutput").ap()

    with tile.TileContext(nc) as tc:
        import contextlib
        ctx = contextlib.ExitStack()
        with ctx:
            consts = ctx.enter_context(tc.tile_pool(name="consts", bufs=1))
            sca = ctx.enter_context(tc.tile_pool(name="sca", bufs=1))
            gp = ctx.enter_context(tc.tile_pool(name="gp", bufs=2))
            vp = ctx.enter_context(tc.tile_pool(name="vp", bufs=2))
            idxp = ctx.enter_context(tc.tile_pool(name="idxp", bufs=2))
            vps = ctx.enter_context(
                tc.tile_pool(name="vps", bufs=1, space="PSUM"))
            ips = ctx.enter_context(
                tc.tile_pool(name="ips", bufs=2, space="PSUM"))

            cIDENT = consts.tile([128, 128], F32, tag="ident")
            nc.sync.dma_start(cIDENT[:], ident[:])
            cXA = consts.tile([128, 16, 8, 2], F32, tag="xa")
            nc.sync.dma_start(cXA[:], xa[:])
            cD0 = consts.tile([128, M0], F32, tag="d0")
            nc.sync.dma_start(cD0[:], d0[:])
            cDZ = consts.tile([128, R1], F32, tag="dz")
            # big table on the Activation HWDGE queue, in parallel with SP
            for ch in range(4):
                sl = slice(ch * (R1 // 4), (ch + 1) * (R1 // 4))
                nc.scalar.dma_start(cDZ[:, sl], dz[:, sl])

            SM = [128, 16, 8]   # sample-major [u, r0, c]

            def st(tag):
                return sca.tile(SM, F32, tag=tag, name=tag)

            def cmaj(tag):
                """c-major index tile + its sample-major [u, r0, c] view."""
                t = sca.tile([128, 8, 16], F32, tag=tag, name=tag)
                return t, t[:].rearrange("p c r -> p r c")

            def ts(out, in_, s1, s2, op1, op2):
                nc.vector.tensor_scalar(out, in_, s1, s2, op1, op2)

            def tt(out, a, b, op):
                nc.vector.tensor_tensor(out, a, b, op)

            # ---- gather wave: idx (sample-major fp32 [u, r0, c]) ->
            #      V [128, 16, 128] fp32 sample-major values
            def wave(idxf, data, num_elems, vtag, wtag):
                """idxf: c-major [128, 8, 16] fp32 index tile."""
                ipt = ips.tile([128, 128], F32, tag="ipt", name=f"ipt_{wtag}")
                nc.tensor.transpose(
                    ipt[:], idxf[:].rearrange("p c r -> p (c r)"), cIDENT[:])
                idx = idxp.tile([128, 128], I16, tag="idx", name=f"idx_{wtag}")
                nc.vector.tensor_copy(idx[:], ipt[:])
                g = gp.tile([128, 2048], F32, tag="g", name=f"g_{wtag}")
                nc.gpsimd.ap_gather(g[:], data[:], idx[:], channels=128,
                                    num_elems=num_elems, d=1, num_idxs=2048)
                v = vp.tile([128, 16, 128], F32, tag=vtag,
                            name=f"v_{wtag}", bufs=1 if vtag == "v_w0" else 2)
                vflat = v[:].rearrange("p r u -> p (r u)")
                gv = g[:].rearrange("p (u r) -> p r u", r=16)
                for b in range(4):
                    ps = vps.tile([128, 512], F32, tag=f"vps{b}",
                                  name=f"vps{b}_{wtag}")
                    for k in range(4):
                        r0 = 4 * b + k
                        nc.tensor.transpose(ps[:, 128 * k:128 * k + 128],
                                            gv[:, r0, :], cIDENT[:])
                    nc.scalar.copy(vflat[:, 512 * b:512 * b + 512], ps[:])
                return v

            # ---------------- stage U: dim-0 chain + per-step grid indices
            x0 = cXA[:, :, :, 0]
            xc0 = st("xc0")
            ts(xc0[:], x0, LEFT, RIGHT, ALU.max, ALU.min)
            m0 = sca.tile(SM, U8, tag="m0")
            tt(m0[:], xc0[:], x0, ALU.is_equal)
            u0 = st("u0")
            ts(u0[:], x0, -X0LO, M0 / (X0HI - X0LO), ALU.add, ALU.mult)
            u0g, u0gv = cmaj("u0g")
            ts(u0gv, u0[:], M0 - 1.51, 0.0, ALU.min, ALU.max)
            j0i = sca.tile(SM, I16, tag="j0i")
            nc.vector.tensor_copy(j0i[:], u0gv)
            j0f = st("j0f")
            nc.vector.tensor_copy(j0f[:], j0i[:])
            fr0 = st("fr0")
            tt(fr0[:], u0gv, j0f[:], ALU.subtract)
            v0 = wave(u0g, cD0, M0, "v_w0", "w0")
            v0v = v0[:].rearrange("p r (c v) -> p r c v", v=16)

            jb256, jb32, fracs = [], [], []
            z0prev = None
            for s in range(FLOW_LENGTH):
                ys = st(f"y0_{s}")
                tt(ys[:], v0v[:, :, :, 2 * s + 1], fr0[:], ALU.mult)
                tt(ys[:], ys[:], v0v[:, :, :, 2 * s], ALU.add)
                z0n = st(f"z0_{s + 1}")
                nc.vector.tensor_copy(z0n[:], x0)
                nc.vector.copy_predicated(z0n[:], m0[:], ys[:])
                nc.sync.dma_start(z0out[s], z0n[:])
                # grid indices for dim-1 step s use z0^(s) (pre-update)
                zsrc = x0 if s == 0 else z0prev[:]
                us = st(f"u_{s}")
                ts(us[:], zsrc, -GLO, (M - 1) / (GHI - GLO), ALU.add, ALU.mult)
                ts(us[:], us[:], M - 1.51, 0.0, ALU.min, ALU.max)
                ji = sca.tile(SM, I16, tag=f"ji_{s}", name=f"ji_{s}")
                nc.vector.tensor_copy(ji[:], us[:])
                jf = st(f"jf_{s}")
                nc.vector.tensor_copy(jf[:], ji[:])
                fs = st(f"fr_{s}")
                tt(fs[:], us[:], jf[:], ALU.subtract)
                fracs.append(fs)
                jb1 = st(f"jb256_{s}")
                nc.vector.tensor_scalar_mul(jb1[:], jf[:], float(M1))
                jb256.append(jb1)
                z0prev = z0n

            # ---------------- stage Z1: sequential dim-1 chain
            z1cur = cXA[:, :, :, 1]
            for s in range(FLOW_LENGTH):
                xc = st("z_xc")
                ts(xc[:], z1cur, LEFT, RIGHT, ALU.max, ALU.min)
                mz = sca.tile(SM, U8, tag="z_mz", name=f"mz_{s}")
                tt(mz[:], xc[:], z1cur, ALU.is_equal)
                q = st("z_q")
                ts(q[:], xc[:], 5.0, M1 / 10.0, ALU.add, ALU.mult)
                ts(q[:], q[:], M1 - 0.51, 0.0, ALU.min, ALU.max)
                i1f, i1fv = cmaj("z_i1f")
                tt(i1fv, jb256[s][:], q[:], ALU.add)
                v1 = wave(i1f, cDZ, R1, "v_t1", f"t1_{s}")
                v1v = v1[:].rearrange("p r (c v) -> p r c v", v=16)
                frb = fracs[s][:].unsqueeze(3)

                c4 = sca.tile([128, 16, 8, 4], F32, tag="z_c4",
                              name=f"c4_{s}")
                tt(c4[:], v1v[:, :, :, 4:8], v1v[:, :, :, 0:4], ALU.subtract)
                tt(c4[:], c4[:], frb.broadcast_to((128, 16, 8, 4)), ALU.mult)
                tt(c4[:], c4[:], v1v[:, :, :, 0:4], ALU.add)

                num = st("z_num")
                tt(num[:], c4[:, :, :, 1], xc[:], ALU.mult)
                tt(num[:], num[:], c4[:, :, :, 0], ALU.add)
                den = st("z_den")
                tt(den[:], c4[:, :, :, 3], xc[:], ALU.mult)
                tt(den[:], den[:], c4[:, :, :, 2], ALU.add)
                rden = st("z_rden")
                nc.vector.reciprocal_approx_fast(rden[:], den[:])
                y = st("z_y")
                tt(y[:], num[:], rden[:], ALU.mult)

                zn = st(f"z1_{s + 1}")
                nc.vector.tensor_copy(zn[:], z1cur)
                nc.vector.copy_predicated(zn[:], mz[:], y[:])
                nc.sync.dma_start(z1out[s], zn[:])
                z1cur = zn[:]

    nc.compile()
    return nc


_NC_CACHE = {}


def _get_program(ns):
    if ns not in _NC_CACHE:
        _NC_CACHE[ns] = _build_program(ns)
    return _NC_CACHE[ns]


def _make_inputs(x, W1, b1, W2, b2, W3, b3, ns):
    D0, DZ = _build_tables(W1, b1, W2, b2, W3, b3)
    shared = dict(
        D0=np.tile(D0, (8, 1)),
        DZ=np.tile(DZ, (8, 1)),
        IDENT=np.eye(128, dtype=np.float32),
    )
    n_cores = x.shape[0] // ns
    in_maps = []
    for n in range(n_cores):
        xs = x[n * ns:(n + 1) * ns]          # [16384, 2]
        # XA[u, r0, c, d] = xs[c*2048 + u*16 + r0, d]
        xa = xs.reshape(8, 128, 16, 2).transpose(1, 2, 0, 3).copy()
        in_maps.append(dict(XA=xa, **shared))
    return in_maps


def _run(x, W1, b1, W2, b2, W3, b3, ns, trace=False):
    from concourse.bass_utils import run_bass_kernel_spmd

    n_cores = x.shape[0] // ns
    nc = _get_program(ns)
    in_maps = _make_inputs(x, W1, b1, W2, b2, W3, b3, ns)
    res = run_bass_kernel_spmd(nc, in_maps, list(range(n_cores)), trace=trace)

    n = x.shape[0]
    zs = np.empty((FLOW_LENGTH + 1, n, 2), np.float32)
    zs[0] = x
    for c in range(n_cores):
        r = res.results[c]
        lo = c * ns
        for s in range(FLOW_LENGTH):
            # [u, r0, cc] -> flat index cc*2048 + u*16 + r0
            zs[s + 1, lo:lo + ns, 0] = r["Z0OUT"][s].transpose(2, 0, 1).reshape(ns)
            zs[s + 1, lo:lo + ns, 1] = r["Z1OUT"][s].transpose(2, 0, 1).reshape(ns)
    return zs, res


def kernel(x, W1, b1, W2, b2, W3, b3):
    x = np.ascontiguousarray(np.asarray(x, dtype=np.float32))
    zs, _ = _run(x, np.asarray(W1, np.float64), np.asarray(b1, np.float64),
                 np.asarray(W2, np.float64), np.asarray(b2, np.float64),
                 np.asarray(W3, np.float64), np.asarray(b3, np.float64),
                 NS)
    return zs
